# revision 7
# baseline (speedup 1.0000x reference)
"""Two-layer weighted GraphSAGE on 8 Trainium2 NeuronCores.

Sharding: nodes are assigned to 8 cores x 98 blocks x 128 slots, degree
balanced within each mod-4 residue class (so the dma_gather class j = src%4
is identical for both layers).  Each core owns the edges whose destination is
local; per (block, class) the edges get a static 5-tile (640 slot) chunk.

Per layer, per core:
  - self path:  out_self = [x | 1].T-major @ [W_s.T ; b] (fp32 matmul,
    bias folded in as a 65th contraction row)
  - messages:   dma_gather x[src] (f32, 256B rows) in 4480-index calls,
    weight+cast to bf16 on DVE, one-hot S built on-chip (is_equal vs iota),
    per-tile bf16 matmul aggT[64,128] += Xgw.T @ S accumulated in PSUM
  - combine:    out = out_self + aggT.T @ Wn.T (+relu for layer 1)
Between layers: AllGather of h (f32) so every core holds the full h table.
"""
import sys
sys.path.insert(0, '/opt/trn_rl_repo')

import numpy as np

N = 100000
E = 1600000
D = 64
P = 8                     # cores
B = 98                    # blocks per core
NLOC = B * 128            # 12544 node slots per core
NPAD = P * NLOC           # 100352 global padded nodes
TQ = 5                    # tiles per (block, class) chunk
CAP = TQ * 128            # 640 edge slots per chunk
TB = 4 * TQ               # 20 tiles per block
G = 7                     # blocks per gather group
NG = B // G               # 14 groups
SEG = G * CAP             # 4480 indices per dma_gather call
KCLS = NPAD // 4          # 25088 rows per class table

_CACHE = {}


def _build_module(mode="full"):
    """mode: full | l1 (layer1 only, h -> out_pi) | l1ng (layer1, no gather)"""
    import concourse.bass as bass
    import concourse.bacc as bacc
    import concourse.tile as tile
    from concourse import mybir
    from concourse.masks import make_identity

    f32 = mybir.dt.float32
    bf16 = mybir.dt.bfloat16
    i16 = mybir.dt.int16

    nc = bacc.Bacc(None, target_bir_lowering=False, debug=False)

    # ---- inputs (per core) ----
    x_pad = nc.dram_tensor("x_pad", [NPAD, D], f32, kind="ExternalInput")
    xT = nc.dram_tensor("xT", [65, NLOC], f32, kind="ExternalInput")
    idx1 = nc.dram_tensor("idx1", [NG, 128, 4 * SEG // 16], i16, kind="ExternalInput")
    idx2 = nc.dram_tensor("idx2", [NG, 128, 4 * SEG // 16], i16, kind="ExternalInput")
    dstm = nc.dram_tensor("dstm", [128, B * TB], i16, kind="ExternalInput")
    w1 = nc.dram_tensor("w1", [128, B * TB], f32, kind="ExternalInput")
    w2 = nc.dram_tensor("w2", [128, B * TB], f32, kind="ExternalInput")
    ws1 = nc.dram_tensor("ws1", [65, D], f32, kind="ExternalInput")   # [Ws1.T+bn; b1]
    ws2 = nc.dram_tensor("ws2", [65, D], f32, kind="ExternalInput")
    wn1 = nc.dram_tensor("wn1", [D, D], bf16, kind="ExternalInput")   # Wn1.T bf16
    wn2 = nc.dram_tensor("wn2", [D, D], bf16, kind="ExternalInput")
    out_pi = nc.dram_tensor("out_pi", [NLOC, D], f32, kind="ExternalOutput")

    # ---- internal DRAM ----
    h_loc = nc.dram_tensor("h_loc", [NLOC, D], f32)
    hT_dram = nc.dram_tensor("hT_dram", [65, NLOC], f32)
    h_full = nc.dram_tensor("h_full", [NPAD, D], f32, addr_space="Shared")

    with tile.TileContext(nc) as tc:
        with (
            tc.tile_pool(name="const", bufs=1) as cst,
            tc.tile_pool(name="idxp", bufs=2) as idxp,
            tc.tile_pool(name="xgp", bufs=2) as xgp,
            tc.tile_pool(name="xgwp", bufs=2) as xgwp,
            tc.tile_pool(name="sp", bufs=3) as sp,
            tc.tile_pool(name="slab", bufs=2) as slab,
            tc.tile_pool(name="small", bufs=4) as small,
            tc.tile_pool(name="psA", bufs=2, space="PSUM") as psA,
            tc.tile_pool(name="psB", bufs=2, space="PSUM") as psB,
            tc.tile_pool(name="psC", bufs=2, space="PSUM") as psC,
        ):
            # constants
            iota_sb = cst.tile([128, 128], i16)
            nc.gpsimd.iota(iota_sb[:], pattern=[[1, 128]], base=0,
                           channel_multiplier=0)
            ident = cst.tile([128, 128], f32)
            make_identity(nc, ident[:])
            dst_sb = cst.tile([128, B * TB], i16)
            nc.sync.dma_start(dst_sb[:], dstm[:])
            w1_sb = cst.tile([128, B * TB], f32, tag="w1c")
            w2_sb = cst.tile([128, B * TB], f32, tag="w2c")
            w_sb = {1: w1_sb, 2: w2_sb}
            nc.sync.dma_start(w_sb[1][:], w1[:])
            nc.sync.dma_start(w_sb[2][:], w2[:])
            ws1_sb = cst.tile([65, D], f32, tag="ws1c")
            ws2_sb = cst.tile([65, D], f32, tag="ws2c")
            ws_sb = {1: ws1_sb, 2: ws2_sb}
            nc.sync.dma_start(ws_sb[1][:], ws1[:])
            nc.sync.dma_start(ws_sb[2][:], ws2[:])
            wn1_sb = cst.tile([D, D], bf16, tag="wn1c")
            wn2_sb = cst.tile([D, D], bf16, tag="wn2c")
            wn_sb = {1: wn1_sb, 2: wn2_sb}
            nc.sync.dma_start(wn_sb[1][:], wn1[:])
            nc.sync.dma_start(wn_sb[2][:], wn2[:])

            def layer(l, table, selfT_dram, idx_dram):
                """Emit one GraphSAGE layer."""
                tcls = table[:].rearrange("(k j) d -> k (j d)", j=4)
                for g in range(NG):
                    idx_sb = idxp.tile([128, 4 * SEG // 16], i16, tag="idx")
                    nc.sync.dma_start(idx_sb[:], idx_dram[g])
                    selfT_sb = slab.tile([65, G * 128], f32, tag="selfT")
                    nc.sync.dma_start(
                        selfT_sb[:], selfT_dram[:, g * G * 128:(g + 1) * G * 128])
                    # self path for the group's blocks
                    outslab = slab.tile([128, G, D], f32, tag="outslab")
                    for bg in range(G):
                        ps_self = psB.tile([128, D], f32, space="PSUM",
                                           tag="ps_self")
                        nc.tensor.matmul(
                            ps_self[:], lhsT=selfT_sb[:, bg * 128:(bg + 1) * 128],
                            rhs=ws_sb[l][:], start=True, stop=True)
                        nc.vector.tensor_copy(outslab[:, bg, :], ps_self[:])
                    # gathers + weighting for the group (class-major)
                    xgw_g = xgwp.tile([128, 4, G * TQ, D], bf16, tag="xgw")
                    for j in range(4):
                        xg = xgp.tile([128, G * TQ, D], f32, tag="xg")
                        if mode == "l1ng":
                            nc.gpsimd.memset(xg[:], 0.0)
                        else:
                            nc.gpsimd.dma_gather(
                                out_ap=xg[:],
                                in_ap=tcls[:, j * D:(j + 1) * D],
                                idxs_ap=idx_sb[:, j * (SEG // 16):(j + 1) * (SEG // 16)],
                                num_idxs=SEG,
                                num_idxs_reg=SEG,
                                elem_size=D,
                                elem_step=4 * D,
                                single_packet=False,
                            )
                        nc.vector.tensor_tensor(
                            out=xgw_g[:, j, :, :], in0=xg[:],
                            in1=w_sb[l][:, j * (B * TQ) + g * (G * TQ):
                                        j * (B * TQ) + (g + 1) * (G * TQ)]
                            .broadcast_to([128, G * TQ, D]),
                            op=mybir.AluOpType.mult)
                    # per-block aggregation + combine
                    hslab = slab.tile([128, G, D], f32, tag="hslab")
                    if l == 1:
                        hTslab = slab.tile([65, G * 128], f32, tag="hTslab")
                        nc.gpsimd.memset(hTslab[64:65, :], 1.0)
                    for bg in range(G):
                        b = g * G + bg
                        s_b = sp.tile([128, TB, 128], bf16, tag="s")
                        nc.vector.tensor_tensor(
                            out=s_b[:],
                            in0=dst_sb[:, b * TB:(b + 1) * TB]
                            .broadcast_to([128, TB, 128]),
                            in1=iota_sb[:, None, :].broadcast_to([128, TB, 128]),
                            op=mybir.AluOpType.is_equal)
                        aggT_ps = psA.tile([64, 128], f32, space="PSUM",
                                           tag="aggT")
                        for j in range(4):
                            for t in range(TQ):
                                nc.tensor.matmul(
                                    aggT_ps[:],
                                    lhsT=xgw_g[:, j, bg * TQ + t, :],
                                    rhs=s_b[:, j * TQ + t, :],
                                    start=(j == 0 and t == 0),
                                    stop=(j == 3 and t == TQ - 1))
                        aggT_sb = small.tile([64, 128], bf16, tag="aggT_sb")
                        nc.vector.tensor_copy(aggT_sb[:], aggT_ps[:])
                        ps_out = psB.tile([128, D], f32, space="PSUM",
                                          tag="ps_out")
                        nc.tensor.matmul(ps_out[:], lhsT=aggT_sb[:],
                                         rhs=wn_sb[l][:], start=True, stop=True)
                        # combine: hslab = self + agg  (+relu for layer 1)
                        nc.vector.tensor_add(
                            out=hslab[:, bg, :], in0=ps_out[:],
                            in1=outslab[:, bg, :])
                        if l == 1:
                            nc.vector.tensor_scalar_max(
                                out=hslab[:, bg, :], in0=hslab[:, bg, :],
                                scalar1=0.0)
                            tr_ps = psC.tile([64, 128], f32, space="PSUM",
                                             tag="tr")
                            nc.tensor.transpose(
                                out=tr_ps[:], in_=hslab[:, bg, :],
                                identity=ident[:])
                            nc.vector.tensor_copy(
                                hTslab[:64, bg * 128:(bg + 1) * 128], tr_ps[:])
                    # write group results
                    if l == 1:
                        if mode == "full":
                            nc.sync.dma_start(
                                h_loc[:].rearrange("(b p) d -> p b d", p=128)
                                [:, g * G:(g + 1) * G, :], hslab[:])
                        else:
                            nc.sync.dma_start(
                                out_pi[:].rearrange("(b p) d -> p b d", p=128)
                                [:, g * G:(g + 1) * G, :], hslab[:])
                        nc.sync.dma_start(
                            hT_dram[:, g * G * 128:(g + 1) * G * 128], hTslab[:])
                    else:
                        nc.sync.dma_start(
                            out_pi[:].rearrange("(b p) d -> p b d", p=128)
                            [:, g * G:(g + 1) * G, :], hslab[:])

            layer(1, x_pad, xT, idx1)
            if mode == "full":
                nc.gpsimd.collective_compute(
                    "AllGather", mybir.AluOpType.bypass,
                    replica_groups=[list(range(P))],
                    ins=[h_loc[:]], outs=[h_full[:]])
                layer(2, h_full, hT_dram, idx2)

    nc.compile()
    return nc


def _prep_inputs(x, edge_index, edge_weight, Ws1, bs1, Wn1, bn1, wp1,
                 Ws2, bs2, Wn2, bn2, wp2):
    """Host-side graph partitioning -> per-core input dicts."""
    import ml_dtypes

    x = np.asarray(x, np.float32)
    src = np.asarray(edge_index[0], np.int64)
    dst = np.asarray(edge_index[1], np.int64)
    w = np.asarray(edge_weight, np.float32)

    counts = np.bincount(dst, minlength=N)
    inv = (1.0 / np.maximum(counts, 1)).astype(np.float32)
    wp1v = float(np.asarray(wp1).reshape(-1)[0])
    wp2v = float(np.asarray(wp2).reshape(-1)[0])
    we = {1: (w * inv[dst] * wp1v).astype(np.float32),
          2: (w * inv[dst] * wp2v).astype(np.float32)}

    # --- node -> (global block, slot), degree-balanced per residue class ---
    NB = P * B                       # 784 global blocks
    gblk = np.empty(N, np.int64)
    slot = np.empty(N, np.int64)
    deg = counts
    for c in range(4):
        nodes = np.where(np.arange(N) % 4 == c)[0]
        order = nodes[np.argsort(-deg[nodes], kind="stable")]
        m = len(order)
        pos = np.arange(m)
        rounds = pos // NB
        off = pos % NB
        blk = np.where(rounds % 2 == 0, off, NB - 1 - off)   # snake
        gblk[order] = blk
        slot[order] = rounds * 4 + c     # slot keeps q%4 == node%4
    q = gblk * 128 + slot                # pi position
    assert slot.max() < 128
    core_of = gblk // B

    # --- edges -> (core, block, class, position) ---
    e_core = core_of[dst]
    e_blk = gblk[dst] % B
    e_cls = src % 4                      # == q[src] % 4 by construction
    e_dstslot = slot[dst]

    # order edges by (core, block, class)
    okey = np.lexsort((e_cls, e_blk, e_core))
    s_src, s_q, s_slot = src[okey], q[src[okey]], e_dstslot[okey]
    s_core, s_blk, s_cls = e_core[okey], e_blk[okey], e_cls[okey]
    s_we = {l: we[l][okey] for l in (1, 2)}

    # position of each edge within its (core, block, class) chunk
    ckey = (s_core * B + s_blk) * 4 + s_cls
    cnt = np.bincount(ckey, minlength=P * B * 4)
    if cnt.max() > CAP:
        raise RuntimeError(f"chunk overflow: {cnt.max()} > {CAP}")
    starts = np.concatenate([[0], np.cumsum(cnt)])[:-1]
    within = np.arange(len(okey)) - starts[ckey]

    # flat edge-slot id per core:  (block, class, t, p)
    t_id = within // 128
    p_id = within % 128
    # dst/w arrays: [128, B*TB] block-major col = b*TB + cls*TQ + t
    col_bm = s_blk * TB + s_cls * TQ + t_id
    # w array class-major col = cls*(B*TQ) + g*(G*TQ) + bg*TQ + t
    g_id = s_blk // G
    bg_id = s_blk % G
    col_cm = s_cls * (B * TQ) + g_id * (G * TQ) + bg_id * TQ + t_id
    # gather index position within (g, cls) segment: i = (bg*TQ+t)*128 + p
    seg_i = (bg_id * TQ + t_id) * 128 + p_id

    in_maps = []
    for c in range(P):
        m = s_core == c
        dstm = np.full((128, B * TB), -1, np.int16)
        dstm[p_id[m], col_bm[m]] = s_slot[m]
        wl = {}
        for l in (1, 2):
            a = np.zeros((128, B * TB), np.float32)
            a[p_id[m], col_cm[m]] = s_we[l][m]
            wl[l] = a
        idx = {}
        for l, val in ((1, s_src[m] // 4), (2, s_q[m] // 4)):
            a = np.zeros((NG, 16, 4 * SEG // 16), np.int16)
            a[g_id[m], seg_i[m] % 16,
              s_cls[m] * (SEG // 16) + seg_i[m] // 16] = val
            idx[l] = np.tile(a, (1, 8, 1))
        # pi-ordered node features for the self path
        perm = np.full(NLOC, -1, np.int64)
        qc = q[core_of == c] - c * NLOC
        perm[qc] = np.where(core_of == c)[0]
        xT = np.zeros((65, NLOC), np.float32)
        valid = perm >= 0
        xT[:D, valid] = x[perm[valid]].T
        xT[D, :] = 1.0

        in_maps.append({
            "x_pad": np.concatenate(
                [x, np.zeros((NPAD - N, D), np.float32)], axis=0),
            "xT": xT,
            "idx1": idx[1], "idx2": idx[2],
            "dstm": dstm, "w1": wl[1], "w2": wl[2],
            "ws1": np.concatenate(
                [np.asarray(Ws1, np.float32).T,
                 (np.asarray(bs1) + np.asarray(bn1)).reshape(1, D)],
                axis=0).astype(np.float32),
            "ws2": np.concatenate(
                [np.asarray(Ws2, np.float32).T,
                 (np.asarray(bs2) + np.asarray(bn2)).reshape(1, D)],
                axis=0).astype(np.float32),
            "wn1": np.asarray(Wn1, np.float32).T.astype(ml_dtypes.bfloat16),
            "wn2": np.asarray(Wn2, np.float32).T.astype(ml_dtypes.bfloat16),
        })
    return in_maps, q


def _install_ntff_shim():
    """Provide antenv.axon_hooks (missing in this image) so trace=True can
    NTFF-profile through the axon PJRT .so."""
    import contextlib
    import ctypes
    import types

    if "antenv.axon_hooks" in sys.modules:
        return
    hook = None
    try:
        lib = ctypes.CDLL("/opt/axon/libaxon_pjrt.so")
        if hasattr(lib, "axon_start_nrt_profile"):
            lib.axon_start_nrt_profile.argtypes = [
                ctypes.POINTER(ctypes.c_int64), ctypes.c_size_t]
            lib.axon_start_nrt_profile.restype = ctypes.c_int64
            lib.axon_stop_nrt_profile.argtypes = [ctypes.c_char_p]
            lib.axon_stop_nrt_profile.restype = ctypes.c_int64

            @contextlib.contextmanager
            def _hook(output_dir, device_ids):
                import jax
                jax.devices()
                if device_ids:
                    ids = (ctypes.c_int64 * len(device_ids))(*device_ids)
                    rc = lib.axon_start_nrt_profile(ids, len(device_ids))
                else:
                    rc = lib.axon_start_nrt_profile(None, 0)
                if rc != 0:
                    raise RuntimeError(f"axon_start_nrt_profile rc={rc}")
                try:
                    yield
                finally:
                    n = lib.axon_stop_nrt_profile(str(output_dir).encode())
                    print(f"ntff profile: {n} file(s) -> {output_dir}",
                          file=sys.stderr)

            hook = _hook
    except OSError:
        pass
    mod = types.ModuleType("antenv.axon_hooks")
    mod.get_axon_ntff_profile_hook = lambda: hook
    mod.set_axon_ntff_profile_hook = lambda h: None
    sys.modules["antenv.axon_hooks"] = mod


def kernel(**inputs):
    _install_ntff_shim()
    from concourse.bass_utils import run_bass_kernel_spmd

    mode = __import__("os").environ.get("GNN_MODE", "full")
    if _CACHE.get("mode") != mode:
        _CACHE["nc"] = _build_module(mode)
        _CACHE["mode"] = mode
    nc = _CACHE["nc"]

    in_maps, q = _prep_inputs(**inputs)
    res = run_bass_kernel_spmd(nc, in_maps, core_ids=list(range(P)),
                               trace=bool(int(__import__("os").environ.get(
                                   "GNN_TRACE", "0"))))
    _CACHE["last_result"] = res
    out_all = np.concatenate([res.results[c]["out_pi"] for c in range(P)],
                             axis=0)
    return out_all[q].astype(np.float32)
